# revision 92
# baseline (speedup 1.0000x reference)
"""ClinicalSafetyLoss Trainium2 kernel (bf16 planes, three-engine split).

loss = CE + 0.3*safety_penalty + 0.5*critical_penalty over outputs [B,3] f32
/ targets [B] i64, B = 4_194_304, data-parallel over 8 NeuronCores.

Staging (host): x is rounded to bf16 and laid out per tile as class planes
[P, 3, K] per core (pure layout/dtype change; all arithmetic stays on
device; each partition's slice is one contiguous run for DMA). t is staged
as bf16 (values {0,1,2} exact). This halves HBM traffic and puts every
stock 2-src vector op in 2x (packed bf16) mode, tensor_scalar in 4x.

Device math per row (x0,x1,x2 logits, t target), split across engines:
  DVE (critical engine):
    m0 = [t<1], m2 = [t>=2]                        [TS 4x]
    d01 = x0-x1, d12 = x2-x1                       [TT paged 2x]
    xa = m0*d01, xb = m2*d12                       [TT 2x]
    g  = q - 2, q = 2 - pred (exact first-max):    [custom, 1x]
         q = select(d01>=0, 2*[d01>=d12], [d12<=0])
    pen = relu(-(t+g)) + relu(5*t*(t+g))           [custom, accum -> Spen]
         == PENALTY_MATRIX[t, pred] exactly
    S = e01 + e12                                  [TT 2x]
  ACT:
    ee = exp(dd);  LL = ln(S + 1)                  [accum -> SLL]
    Msign = sign(1 - (2/15)pen), M = (B - sum)/2   [accum; +1 iff pen < 7.5]
  PE (ones-vector column-sum matmuls into two PSUM banks):
    X  = sum(xa) + sum(xb);  G2 = sum(m2)

Host combine (f64): ce = (SLL - X)/B, crit = 10*M/max(G2,1),
loss = ce + 0.3*Spen/B + crit.
"""

import numpy as np

B_TOTAL = 4_194_304
N_CORES = 8
BC = B_TOTAL // N_CORES          # rows per core = 524_288
P = 128                          # SBUF partitions
KTOT = BC // P                   # rows per partition = 4096
K_SCHED = [1024, 2048, 1024]     # ramped; small last tile shortens the tail
T = len(K_SCHED)

N_DVE = 1                        # pen
N_ACT = 2                        # SLL, Msign
MM_N = 512                       # psum bank columns for the X matmul-reduce

_STATE: dict = {}


def _register_dve_ops():
    """Register the two fused vector-engine ops (runtime append to the
    custom-DVE registry; sha computed locally so compile's drift check
    passes)."""
    import concourse.dve_ops as dvo
    from concourse.dve_spec import (
        Spec, Src0, Src1, Zero, C0, C1, select, maxx, lower,
    )
    from concourse.dve_spec import _has_src1
    from concourse.dve_uop import DveOpSpec
    from operator import add

    def mk(name, spec, subdim=False):
        for o in dvo.OPS:
            if o.name == name:
                return o
        shas = {}
        for ver in ("v3", "v4"):
            uops = lower(spec, ver=ver)
            shas[ver] = DveOpSpec(
                name=name, opcode=0, uops=uops, rd1_en=_has_src1(spec)
            ).sha(ver)
        op = dvo.DveOp(name, spec, subdim=subdim, uops_sha=shas)
        dvo.OPS.append(op)
        dvo.CUSTOM_DVE_SPECS[name] = spec
        dvo._SUB_OPCODE_FOR_NAME[name] = dvo._CUSTOM_DVE_ROW_BASE + len(dvo.OPS) - 1
        return op

    # g = q - 2, q = 2-pred (exact first-max argmax):
    #   q = select(d01>=0, 2*[d01>=d12], [d12<=0]);  in0=d01, in1=d12,
    #   s0 = s1 = 2.0
    def _qg_ref(in0, in1, s0, s1, imm2):
        q = np.where(in0 >= 0, s0 * (in0 >= in1), (in1 <= 0).astype(np.float32))
        return (q - s1).astype(np.float32)

    op_qg = mk("CSL_QG", Spec(
        body=select(Src0 >= Zero, (Src0 >= Src1) * C0, Src1 <= Zero) - C1,
        reference=_qg_ref,
    ))

    # pen = relu(-(t+g)) + relu(5*t*(t+g));  in0=t, in1=g, s1=5.0; accum add
    def _pen_ref(in0, in1, s0, s1, imm2):
        w = in0 + in1
        b = np.maximum(-w, 0.0) + np.maximum(s1 * in0 * w, 0.0)
        b = b.astype(np.float32)
        return b, b.reshape(b.shape[0], -1).sum(axis=-1, keepdims=True)

    _w = Src0 + Src1
    op_pen = mk("CSL_PEN", Spec(
        body=maxx(Zero - _w, Zero) + maxx(_w * Src0 * C1, Zero),
        accum=add,
        reference=_pen_ref,
    ))
    return op_qg, op_pen


def _build():
    """Trace + compile the per-core Bass program. Returns the finalized nc."""
    import concourse.bacc as bacc
    import concourse.mybir as mybir
    import concourse.tile as tile

    op_qg, op_pen = _register_dve_ops()

    f32 = mybir.dt.float32
    bf16 = mybir.dt.bfloat16
    Alu = mybir.AluOpType
    Act = mybir.ActivationFunctionType

    nc = bacc.Bacc("TRN2", target_bir_lowering=False, debug=False)

    # Pin Exp/Ln/Relu to the one ACT table set that holds them all
    # (natural_log_exp_and_others) so the per-tile func mix doesn't thrash
    # ACT_TABLE_LOADs.
    from concourse.hw_specs import get_activation_tables
    tabs = get_activation_tables(nc.m.arch)
    for name, funcs in tabs.items():
        if name != "natural_log_exp_and_others":
            for fn in (Act.Exp, Act.Ln, Act.Identity, Act.Square, Act.Copy,
                       Act.Relu, Act.Sign):
                funcs.discard(fn)

    x_drams = [nc.dram_tensor(f"x{i}", [P, 3, K], bf16, kind="ExternalInput")
               for i, K in enumerate(K_SCHED)]
    t_drams = [nc.dram_tensor(f"t{i}", [P, 1, K], bf16, kind="ExternalInput")
               for i, K in enumerate(K_SCHED)]
    acc_dve_dram = nc.dram_tensor("acc_dve", [P, T * N_DVE], f32,
                                  kind="ExternalOutput")
    acc_act_dram = nc.dram_tensor("acc_act", [P, T * N_ACT], f32,
                                  kind="ExternalOutput")
    xsum_dram = nc.dram_tensor("xsum", [1, 2 * MM_N], f32, kind="ExternalOutput")

    assert sum(K_SCHED) == KTOT

    with tile.TileContext(nc) as tc:
        with (
            tc.tile_pool(name="xin", bufs=2) as xpool,
            tc.tile_pool(name="tin", bufs=2) as tpool,
            tc.tile_pool(name="work", bufs=2) as wpool,
            tc.tile_pool(name="accp", bufs=1) as apool,
            tc.tile_pool(name="ps", bufs=1, space="PSUM") as pspool,
        ):
            acc_dve = apool.tile([P, T * N_DVE], f32, tag="acc_dve")
            acc_act = apool.tile([P, T * N_ACT], f32, tag="acc_act")
            xps = pspool.tile([1, MM_N], f32, tag="xps")
            g2ps = pspool.tile([1, MM_N], f32, tag="g2ps")
            ones = nc.const_aps.tensor(1.0, (P, 1), bf16)
            n_chunks = sum((K + MM_N - 1) // MM_N for K in K_SCHED)
            n_mm = 2 * n_chunks
            mm_i = 0
            mm2_i = 0

            # Serial (t, x) DMA issue per tile implicitly prioritizes the
            # lead tile's data; parallel rings or upfront issue measurably
            # starve the first x transfer.
            for it, K in enumerate(K_SCHED):
                xt = xpool.tile([P, 3, K], bf16, tag="x")
                tt = tpool.tile([P, 1, K], bf16, tag="t")
                if it == 0:
                    # lead tile: x first — dd gates the whole pipeline
                    nc.sync.dma_start(xt[:], x_drams[it][:])
                    nc.sync.dma_start(tt[:], t_drams[it][:])
                else:
                    nc.sync.dma_start(tt[:], t_drams[it][:])
                    nc.sync.dma_start(xt[:], x_drams[it][:])
                tl = tt[:, 0, :]

                ad = lambda q: acc_dve[:, it * N_DVE + q: it * N_DVE + q + 1]
                aa = lambda q: acc_act[:, it * N_ACT + q: it * N_ACT + q + 1]

                # masks first: they only need t, which lands early
                m0 = wpool.tile([P, K], bf16, tag="m0")
                nc.vector.tensor_scalar(m0[:], tl, 1.0, None, Alu.is_lt)
                m2 = wpool.tile([P, K], bf16, tag="m2")
                nc.vector.tensor_scalar(m2[:], tl, 2.0, None, Alu.is_ge)

                # dd[:,0,:] = x0-x1, dd[:,1,:] = x2-x1: page AP walks the
                # (x0, x2) planes, in1 broadcasts the x1 plane over both.
                x02 = xt[:, 0:3:2, :]
                x11 = xt[:, 1:2, :].to_broadcast([P, 2, K])
                dd = wpool.tile([P, 2, K], bf16, tag="dd")
                nc.vector.tensor_tensor(dd[:], x02, x11, Alu.subtract)
                d01 = dd[:, 0, :]
                d12 = dd[:, 1, :]

                # --- exp on ACT overlapped with DVE mask work ---
                ee = wpool.tile([P, 2, K], bf16, tag="ee")
                nc.scalar.activation(ee[:], dd[:], Act.Exp)

                def emit_products():
                    xa = wpool.tile([P, K], bf16, tag="xa")
                    nc.vector.tensor_tensor(xa[:], m0[:], d01, Alu.mult)
                    xb = wpool.tile([P, K], bf16, tag="xb")
                    nc.vector.tensor_tensor(xb[:], m2[:], d12, Alu.mult)
                    return xa, xb, m2

                def emit_pen():
                    g = wpool.tile([P, K], bf16, tag="g")
                    nc.vector._custom_dve(op_qg, out=g[:], in0=d01, in1=d12,
                                          s0=2.0, s1=2.0)
                    pen = wpool.tile([P, K], bf16, tag="pen")
                    nc.vector._custom_dve(op_pen, out=pen[:], in0=tl,
                                          in1=g[:], s1=5.0, accum_out=ad(0))
                    # miss via sign(1 - (2/15) pen): +1 iff pen < 7.5
                    mt = wpool.tile([P, K], bf16, tag="mt")
                    nc.scalar.activation(mt[:], pen[:], Act.Sign, bias=1.0,
                                         scale=-2.0 / 15.0, accum_out=aa(1))

                xa, xb, m2 = emit_products()
                for prod in (xa, xb):
                    for c in range(0, K, MM_N):
                        n = min(MM_N, K - c)
                        nc.tensor.matmul(xps[:, 0:n], ones, prod[:, c:c + n],
                                         start=(mm_i == 0),
                                         stop=(mm_i == n_mm - 1))
                        mm_i += 1
                for c in range(0, K, MM_N):
                    n = min(MM_N, K - c)
                    nc.tensor.matmul(g2ps[:, 0:n], ones, m2[:, c:c + n],
                                     start=(mm2_i == 0),
                                     stop=(mm2_i == n_chunks - 1))
                    mm2_i += 1
                emit_pen()

                # --- CE path ---
                S = wpool.tile([P, K], bf16, tag="S")
                nc.vector.tensor_tensor(S[:], ee[:, 0, :], ee[:, 1, :], Alu.add)
                LL = wpool.tile([P, K], bf16, tag="LL")
                nc.scalar.activation(LL[:], S[:], Act.Ln, bias=1.0,
                                     accum_out=aa(0))

                # Stream this tile's accumulators out now so the kernel tail
                # only waits on the last tile's columns.
                nc.sync.dma_start(
                    acc_dve_dram[:, it * N_DVE:(it + 1) * N_DVE],
                    acc_dve[:, it * N_DVE:(it + 1) * N_DVE])
                nc.sync.dma_start(
                    acc_act_dram[:, it * N_ACT:(it + 1) * N_ACT],
                    acc_act[:, it * N_ACT:(it + 1) * N_ACT])

            xs = apool.tile([1, 2 * MM_N], f32, tag="xs")
            nc.vector.tensor_copy(xs[:, 0:MM_N], xps[:])
            nc.vector.tensor_copy(xs[:, MM_N:2 * MM_N], g2ps[:])
            nc.sync.dma_start(xsum_dram[:], xs[:])

    nc.compile()
    return nc


def _ensure_built():
    if "nc" not in _STATE:
        _STATE["nc"] = _build()
    return _STATE["nc"]


def _combine(results):
    """Host-side float64 combine of the per-core accumulators into the loss."""
    tot_dve = np.zeros(N_DVE, dtype=np.float64)
    tot_act = np.zeros(N_ACT, dtype=np.float64)
    X = 0.0
    G2 = 0.0
    for r in results:
        tot_dve += r["acc_dve"].astype(np.float64).reshape(P, T, N_DVE).sum(axis=(0, 1))
        tot_act += r["acc_act"].astype(np.float64).reshape(P, T, N_ACT).sum(axis=(0, 1))
        xsum = r["xsum"].astype(np.float64).reshape(2, MM_N)
        X += xsum[0].sum()
        G2 += xsum[1].sum()
    Spen, = tot_dve
    SLL, Msgn = tot_act

    B = float(B_TOTAL)
    M = (B - Msgn) / 2.0
    ce_sum = SLL - X
    critical = 10.0 * M / max(G2, 1.0) if G2 > 0 else 0.0
    loss = ce_sum / B + 0.3 * Spen / B + critical
    return np.asarray(loss, dtype=np.float32)


def kernel(outputs: np.ndarray, targets: np.ndarray) -> np.ndarray:
    import os
    import ml_dtypes
    from concourse.bass_utils import run_bass_kernel_spmd

    nc = _ensure_built()
    bf16 = ml_dtypes.bfloat16

    # [B,3] f32 -> per-core, per-tile class planes [P, 3, K] bf16 (pure
    # layout/dtype staging; rounding only, no arithmetic). Each partition's
    # tile slice is one contiguous 6K-12K byte run for fast DMA.
    x = np.asarray(outputs, dtype=np.float32).reshape(
        N_CORES, P, KTOT, 3).astype(bf16)
    tp = np.asarray(targets).astype(bf16).reshape(N_CORES, P, 1, KTOT)

    in_maps = []
    for c in range(N_CORES):
        m = {}
        k_off = 0
        for i, K in enumerate(K_SCHED):
            m[f"x{i}"] = np.ascontiguousarray(
                x[c, :, k_off:k_off + K, :].transpose(0, 2, 1))
            m[f"t{i}"] = np.ascontiguousarray(tp[c, :, :, k_off:k_off + K])
            k_off += K
        in_maps.append(m)
    trace = bool(int(os.environ.get("CSL_TRACE", "0")))
    tmpdir = os.environ.get("CSL_TRACE_DIR") or None
    res = run_bass_kernel_spmd(nc, in_maps, list(range(N_CORES)), trace=trace,
                               tmpdir=tmpdir)
    kernel._last_exec_time_ns = getattr(res, "exec_time_ns", None)
    return _combine(res.results)


kernel._last_exec_time_ns = None


# revision 93
# speedup vs baseline: 1.0596x; 1.0596x over previous
"""ClinicalSafetyLoss Trainium2 kernel (bf16 planes, three-engine split).

loss = CE + 0.3*safety_penalty + 0.5*critical_penalty over outputs [B,3] f32
/ targets [B] i64, B = 4_194_304, data-parallel over 8 NeuronCores.

Staging (host): x is rounded to bf16 and laid out per tile as class planes
[P, 3, K] per core (pure layout/dtype change; all arithmetic stays on
device; each partition's slice is one contiguous run for DMA). t is staged
as bf16 (values {0,1,2} exact). This halves HBM traffic and puts every
stock 2-src vector op in 2x (packed bf16) mode, tensor_scalar in 4x.

Device math per row (x0,x1,x2 logits, t target), split across engines:
  DVE (critical engine):
    m0 = [t<1], m2 = [t>=2]                        [TS 4x]
    d01 = x0-x1, d12 = x2-x1                       [TT paged 2x]
    xa = m0*d01, xb = m2*d12                       [TT 2x]
    g  = q - 2, q = 2 - pred (exact first-max):    [custom, 1x]
         q = select(d01>=0, 2*[d01>=d12], [d12<=0])
    pen = relu(-(t+g)) + relu(5*t*(t+g))           [custom, accum -> Spen]
         == PENALTY_MATRIX[t, pred] exactly
    S = e01 + e12                                  [TT 2x]
  ACT:
    ee = exp(dd);  LL = ln(S + 1)                  [accum -> SLL]
    Msign = sign(1 - (2/15)pen), M = (B - sum)/2   [accum; +1 iff pen < 7.5]
  PE (ones-vector column-sum matmuls into two PSUM banks):
    X  = sum(xa) + sum(xb);  G2 = sum(m2)

Host combine (f64): ce = (SLL - X)/B, crit = 10*M/max(G2,1),
loss = ce + 0.3*Spen/B + crit.
"""

import numpy as np

B_TOTAL = 4_194_304
N_CORES = 8
BC = B_TOTAL // N_CORES          # rows per core = 524_288
P = 128                          # SBUF partitions
KTOT = BC // P                   # rows per partition = 4096
K_SCHED = [1024, 2048, 1024]     # ramped; small last tile shortens the tail
T = len(K_SCHED)

N_DVE = 1                        # pen
N_ACT = 2                        # SLL, Msign
MM_N = 512                       # psum bank columns for the X matmul-reduce

_STATE: dict = {}


def _register_dve_ops():
    """Register the two fused vector-engine ops (runtime append to the
    custom-DVE registry; sha computed locally so compile's drift check
    passes)."""
    import concourse.dve_ops as dvo
    from concourse.dve_spec import (
        Spec, Src0, Src1, Zero, C0, C1, select, maxx, lower,
    )
    from concourse.dve_spec import _has_src1
    from concourse.dve_uop import DveOpSpec
    from operator import add

    def mk(name, spec, subdim=False):
        for o in dvo.OPS:
            if o.name == name:
                return o
        shas = {}
        for ver in ("v3", "v4"):
            uops = lower(spec, ver=ver)
            shas[ver] = DveOpSpec(
                name=name, opcode=0, uops=uops, rd1_en=_has_src1(spec)
            ).sha(ver)
        op = dvo.DveOp(name, spec, subdim=subdim, uops_sha=shas)
        dvo.OPS.append(op)
        dvo.CUSTOM_DVE_SPECS[name] = spec
        dvo._SUB_OPCODE_FOR_NAME[name] = dvo._CUSTOM_DVE_ROW_BASE + len(dvo.OPS) - 1
        return op

    # g = q - 2, q = 2-pred (exact first-max argmax):
    #   q = select(d01>=0, 2*[d01>=d12], [d12<=0]);  in0=d01, in1=d12,
    #   s0 = s1 = 2.0
    def _qg_ref(in0, in1, s0, s1, imm2):
        q = np.where(in0 >= 0, s0 * (in0 >= in1), (in1 <= 0).astype(np.float32))
        return (q - s1).astype(np.float32)

    op_qg = mk("CSL_QG", Spec(
        body=select(Src0 >= Zero, (Src0 >= Src1) * C0, Src1 <= Zero) - C1,
        reference=_qg_ref,
    ))

    # pen = relu(-(t+g)) + relu(5*t*(t+g));  in0=t, in1=g, s1=5.0; accum add
    def _pen_ref(in0, in1, s0, s1, imm2):
        w = in0 + in1
        b = np.maximum(-w, 0.0) + np.maximum(s1 * in0 * w, 0.0)
        b = b.astype(np.float32)
        return b, b.reshape(b.shape[0], -1).sum(axis=-1, keepdims=True)

    _w = Src0 + Src1
    op_pen = mk("CSL_PEN", Spec(
        body=maxx(Zero - _w, Zero) + maxx(_w * Src0 * C1, Zero),
        accum=add,
        reference=_pen_ref,
    ))
    return op_qg, op_pen


def _build():
    """Trace + compile the per-core Bass program. Returns the finalized nc."""
    import concourse.bacc as bacc
    import concourse.mybir as mybir
    import concourse.tile as tile

    op_qg, op_pen = _register_dve_ops()

    f32 = mybir.dt.float32
    bf16 = mybir.dt.bfloat16
    Alu = mybir.AluOpType
    Act = mybir.ActivationFunctionType

    nc = bacc.Bacc("TRN2", target_bir_lowering=False, debug=False)

    # Pin Exp/Ln/Relu to the one ACT table set that holds them all
    # (natural_log_exp_and_others) so the per-tile func mix doesn't thrash
    # ACT_TABLE_LOADs.
    from concourse.hw_specs import get_activation_tables
    tabs = get_activation_tables(nc.m.arch)
    for name, funcs in tabs.items():
        if name != "natural_log_exp_and_others":
            for fn in (Act.Exp, Act.Ln, Act.Identity, Act.Square, Act.Copy,
                       Act.Relu, Act.Sign):
                funcs.discard(fn)

    x_drams = [nc.dram_tensor(f"x{i}", [P, 3, K], bf16, kind="ExternalInput")
               for i, K in enumerate(K_SCHED)]
    t_drams = [nc.dram_tensor(f"t{i}", [P, 1, K], bf16, kind="ExternalInput")
               for i, K in enumerate(K_SCHED)]
    acc_dve_dram = nc.dram_tensor("acc_dve", [P, T * N_DVE], f32,
                                  kind="ExternalOutput")
    acc_act_dram = nc.dram_tensor("acc_act", [P, T * N_ACT], f32,
                                  kind="ExternalOutput")
    xsum_dram = nc.dram_tensor("xsum", [1, 2 * MM_N], f32, kind="ExternalOutput")

    assert sum(K_SCHED) == KTOT

    with tile.TileContext(nc) as tc:
        with (
            tc.tile_pool(name="xin", bufs=2) as xpool,
            tc.tile_pool(name="tin", bufs=2) as tpool,
            tc.tile_pool(name="work", bufs=2) as wpool,
            tc.tile_pool(name="accp", bufs=1) as apool,
            tc.tile_pool(name="ps", bufs=1, space="PSUM") as pspool,
        ):
            acc_dve = apool.tile([P, T * N_DVE], f32, tag="acc_dve")
            acc_act = apool.tile([P, T * N_ACT], f32, tag="acc_act")
            xps = pspool.tile([1, MM_N], f32, tag="xps")
            g2ps = pspool.tile([1, MM_N], f32, tag="g2ps")
            ones = nc.const_aps.tensor(1.0, (P, 1), bf16)
            n_chunks = sum((K + MM_N - 1) // MM_N for K in K_SCHED)
            n_mm = 2 * n_chunks
            mm_i = 0
            mm2_i = 0

            # Serial (t, x) DMA issue per tile implicitly prioritizes the
            # lead tile's data; parallel rings or upfront issue measurably
            # starve the first x transfer.
            for it, K in enumerate(K_SCHED):
                xt = xpool.tile([P, 3, K], bf16, tag="x")
                tt = tpool.tile([P, 1, K], bf16, tag="t")
                nc.sync.dma_start(tt[:], t_drams[it][:])
                nc.sync.dma_start(xt[:], x_drams[it][:])
                tl = tt[:, 0, :]

                ad = lambda q: acc_dve[:, it * N_DVE + q: it * N_DVE + q + 1]
                aa = lambda q: acc_act[:, it * N_ACT + q: it * N_ACT + q + 1]

                # masks first: they only need t, which lands early
                m0 = wpool.tile([P, K], bf16, tag="m0")
                nc.vector.tensor_scalar(m0[:], tl, 1.0, None, Alu.is_lt)
                m2 = wpool.tile([P, K], bf16, tag="m2")
                nc.vector.tensor_scalar(m2[:], tl, 2.0, None, Alu.is_ge)

                # dd[:,0,:] = x0-x1, dd[:,1,:] = x2-x1: page AP walks the
                # (x0, x2) planes, in1 broadcasts the x1 plane over both.
                x02 = xt[:, 0:3:2, :]
                x11 = xt[:, 1:2, :].to_broadcast([P, 2, K])
                dd = wpool.tile([P, 2, K], bf16, tag="dd")
                nc.vector.tensor_tensor(dd[:], x02, x11, Alu.subtract)
                d01 = dd[:, 0, :]
                d12 = dd[:, 1, :]

                # --- exp on ACT overlapped with DVE mask work ---
                ee = wpool.tile([P, 2, K], bf16, tag="ee")
                nc.scalar.activation(ee[:], dd[:], Act.Exp)

                def emit_products():
                    xa = wpool.tile([P, K], bf16, tag="xa")
                    nc.vector.tensor_tensor(xa[:], m0[:], d01, Alu.mult)
                    xb = wpool.tile([P, K], bf16, tag="xb")
                    nc.vector.tensor_tensor(xb[:], m2[:], d12, Alu.mult)
                    return xa, xb, m2

                def emit_pen():
                    g = wpool.tile([P, K], bf16, tag="g")
                    nc.vector._custom_dve(op_qg, out=g[:], in0=d01, in1=d12,
                                          s0=2.0, s1=2.0)
                    pen = wpool.tile([P, K], bf16, tag="pen")
                    nc.vector._custom_dve(op_pen, out=pen[:], in0=tl,
                                          in1=g[:], s1=5.0, accum_out=ad(0))
                    # miss via sign(1 - (2/15) pen): +1 iff pen < 7.5
                    mt = wpool.tile([P, K], bf16, tag="mt")
                    nc.scalar.activation(mt[:], pen[:], Act.Sign, bias=1.0,
                                         scale=-2.0 / 15.0, accum_out=aa(1))

                xa, xb, m2 = emit_products()
                for prod in (xa, xb):
                    for c in range(0, K, MM_N):
                        n = min(MM_N, K - c)
                        nc.tensor.matmul(xps[:, 0:n], ones, prod[:, c:c + n],
                                         start=(mm_i == 0),
                                         stop=(mm_i == n_mm - 1))
                        mm_i += 1
                for c in range(0, K, MM_N):
                    n = min(MM_N, K - c)
                    nc.tensor.matmul(g2ps[:, 0:n], ones, m2[:, c:c + n],
                                     start=(mm2_i == 0),
                                     stop=(mm2_i == n_chunks - 1))
                    mm2_i += 1
                emit_pen()

                # --- CE path ---
                S = wpool.tile([P, K], bf16, tag="S")
                nc.vector.tensor_tensor(S[:], ee[:, 0, :], ee[:, 1, :], Alu.add)
                LL = wpool.tile([P, K], bf16, tag="LL")
                nc.scalar.activation(LL[:], S[:], Act.Ln, bias=1.0,
                                     accum_out=aa(0))

                # Stream this tile's accumulators out now so the kernel tail
                # only waits on the last tile's columns.
                nc.sync.dma_start(
                    acc_dve_dram[:, it * N_DVE:(it + 1) * N_DVE],
                    acc_dve[:, it * N_DVE:(it + 1) * N_DVE])
                nc.sync.dma_start(
                    acc_act_dram[:, it * N_ACT:(it + 1) * N_ACT],
                    acc_act[:, it * N_ACT:(it + 1) * N_ACT])

            xs = apool.tile([1, 2 * MM_N], f32, tag="xs")
            nc.vector.tensor_copy(xs[:, 0:MM_N], xps[:])
            nc.vector.tensor_copy(xs[:, MM_N:2 * MM_N], g2ps[:])
            nc.sync.dma_start(xsum_dram[:], xs[:])

    nc.compile()
    return nc


def _ensure_built():
    if "nc" not in _STATE:
        _STATE["nc"] = _build()
    return _STATE["nc"]


def _combine(results):
    """Host-side float64 combine of the per-core accumulators into the loss."""
    tot_dve = np.zeros(N_DVE, dtype=np.float64)
    tot_act = np.zeros(N_ACT, dtype=np.float64)
    X = 0.0
    G2 = 0.0
    for r in results:
        tot_dve += r["acc_dve"].astype(np.float64).reshape(P, T, N_DVE).sum(axis=(0, 1))
        tot_act += r["acc_act"].astype(np.float64).reshape(P, T, N_ACT).sum(axis=(0, 1))
        xsum = r["xsum"].astype(np.float64).reshape(2, MM_N)
        X += xsum[0].sum()
        G2 += xsum[1].sum()
    Spen, = tot_dve
    SLL, Msgn = tot_act

    B = float(B_TOTAL)
    M = (B - Msgn) / 2.0
    ce_sum = SLL - X
    critical = 10.0 * M / max(G2, 1.0) if G2 > 0 else 0.0
    loss = ce_sum / B + 0.3 * Spen / B + critical
    return np.asarray(loss, dtype=np.float32)


def kernel(outputs: np.ndarray, targets: np.ndarray) -> np.ndarray:
    import os
    import ml_dtypes
    from concourse.bass_utils import run_bass_kernel_spmd

    nc = _ensure_built()
    bf16 = ml_dtypes.bfloat16

    # [B,3] f32 -> per-core, per-tile class planes [P, 3, K] bf16 (pure
    # layout/dtype staging; rounding only, no arithmetic). Each partition's
    # tile slice is one contiguous 6K-12K byte run for fast DMA.
    x = np.asarray(outputs, dtype=np.float32).reshape(
        N_CORES, P, KTOT, 3).astype(bf16)
    tp = np.asarray(targets).astype(bf16).reshape(N_CORES, P, 1, KTOT)

    in_maps = []
    for c in range(N_CORES):
        m = {}
        k_off = 0
        for i, K in enumerate(K_SCHED):
            m[f"x{i}"] = np.ascontiguousarray(
                x[c, :, k_off:k_off + K, :].transpose(0, 2, 1))
            m[f"t{i}"] = np.ascontiguousarray(tp[c, :, :, k_off:k_off + K])
            k_off += K
        in_maps.append(m)
    trace = bool(int(os.environ.get("CSL_TRACE", "0")))
    tmpdir = os.environ.get("CSL_TRACE_DIR") or None
    res = run_bass_kernel_spmd(nc, in_maps, list(range(N_CORES)), trace=trace,
                               tmpdir=tmpdir)
    kernel._last_exec_time_ns = getattr(res, "exec_time_ns", None)
    return _combine(res.results)


kernel._last_exec_time_ns = None
